# revision 1
# baseline (speedup 1.0000x reference)
"""LGCN encoder (3-layer, dual-adjacency message passing) on 8 Trainium2 cores.

Strategy: 1D row partition of the N=150k node dimension across 8 cores.
Each SpMM is computed destination-tile by destination-tile (128 rows):
 - host pre-sorts each core's edges by (dst tile, source window), padded to a
   uniform chunk grid; chunks are 128 edges
 - device gathers source rows with dma_gather (int16 indices relative to a
   static window base; windows cover the padded node table; <=1024 idx/instr)
 - a DVE tensor_scalar builds the selector S[e, r] = (iota[r]==rowloc[e])*val[e]
 - PE accumulates X_g.T @ S into PSUM [feat, rows] over all chunks of the tile
 - the per-layer Linear(2E->E) consumes the [feat, rows] tiles directly as lhsT
Between layers an AllGather shares each core's ego shard. Final user/item
lookup is an on-device gather + indirect scatter; host merges by row ownership.
"""
import sys
import os
import dataclasses

sys.path.insert(0, "/opt/trn_rl_repo")

import numpy as np
import ml_dtypes
from dataclasses import dataclass

import concourse.bass as bass
import concourse.bacc as bacc
import concourse.mybir as mybir
from concourse.tile import TileContext
from concourse import bass_utils

P = 128
E = 128
WIN_MAX = 30720
GB = 8  # max chunks (of 128 idxs) per dma_gather instruction (1024-idx cap)


@dataclass(frozen=True)
class Cfg:
    n_users: int
    n_items: int
    n_cores: int
    shard: int          # real rows per core (n_nodes / n_cores)
    tiles: int          # tile count per core
    nw: int             # chunks (of 128 edges) per (tile, window)
    n_win: int          # source windows
    win: int            # window size in rows (<= 32767)
    nl: int             # layers
    bq: int             # final-lookup slots per core (multiple of 128)
    nb: int             # lookup batch size (users==items batch)
    use_bf16: bool

    @property
    def shard_g(self):
        return self.tiles * P

    @property
    def n_pad(self):
        return self.shard_g * self.n_cores

    @property
    def nb8(self):      # gather batches per (tile, window)
        return -(-self.nw // GB)

    @property
    def slots(self):    # chunk slots per tile per matrix
        return self.n_win * self.nw

    @property
    def dt(self):
        return mybir.dt.bfloat16 if self.use_bf16 else mybir.dt.float32

    @property
    def npdt(self):
        return ml_dtypes.bfloat16 if self.use_bf16 else np.float32

    @property
    def nout(self):
        return 2 * self.nb + 8


def build_nc(cfg: Cfg):
    DT = cfg.dt
    S = cfg.slots
    IL = cfg.n_win * cfg.nb8 * 64        # idx columns per tile

    nc = bacc.Bacc("TRN2", target_bir_lowering=False)

    x0 = nc.dram_tensor("x0", [cfg.n_pad, E], DT, kind="ExternalInput")
    gidx = [
        nc.dram_tensor(f"gidx{m}", [cfg.tiles, P, IL], mybir.dt.int16,
                       kind="ExternalInput")
        for m in range(2)
    ]
    scal = [
        nc.dram_tensor(f"scal{m}", [cfg.tiles, P, S, 2], mybir.dt.float32, kind="ExternalInput")
        for m in range(2)
    ]
    wt = nc.dram_tensor("wt", [P, cfg.nl, 2, E], mybir.dt.float32, kind="ExternalInput")
    bb = nc.dram_tensor("bb", [P, cfg.nl, E], mybir.dt.float32, kind="ExternalInput")
    iota_in = nc.dram_tensor("iota", [P, P], mybir.dt.float32, kind="ExternalInput")
    fidx = nc.dram_tensor("fidx", [P, cfg.bq // 16], mybir.dt.int16, kind="ExternalInput")
    fpos = nc.dram_tensor("fpos", [P, cfg.bq // P], mybir.dt.int32, kind="ExternalInput")
    out_d = nc.dram_tensor("out", [cfg.nout, E], mybir.dt.float32,
                           kind="ExternalOutput")

    rg = [list(range(cfg.n_cores))]

    with TileContext(nc) as tc:
        with (
            tc.tile_pool(name="const", bufs=1) as constp,
            tc.tile_pool(name="idxp", bufs=4) as idxp,
            tc.tile_pool(name="scalp", bufs=6) as scalp,
            tc.tile_pool(name="xga", bufs=3) as xgap,
            tc.tile_pool(name="xgb", bufs=3) as xgbp,
            tc.tile_pool(name="svalp", bufs=8) as svalp,
            tc.tile_pool(name="accp", bufs=6) as accp,
            tc.tile_pool(name="egop", bufs=4) as egop,
            tc.tile_pool(name="psA", bufs=2, space="PSUM") as psA,
            tc.tile_pool(name="psB", bufs=2, space="PSUM") as psB,
            tc.tile_pool(name="psE", bufs=2, space="PSUM") as psE,
            tc.tile_pool(name="dram", bufs=1, space="DRAM") as dramp,
            tc.tile_pool(name="fin", bufs=1) as finp,
        ):
            iota_t = constp.tile([P, P], mybir.dt.float32)
            nc.sync.dma_start(out=iota_t[:], in_=iota_in[:, :])
            w_t = constp.tile([P, cfg.nl, 2, E], mybir.dt.float32)
            nc.sync.dma_start(out=w_t[:], in_=wt[:, :, :, :])
            b_t = constp.tile([P, cfg.nl, E], mybir.dt.float32)
            nc.sync.dma_start(out=b_t[:], in_=bb[:, :, :])

            ego_loc = [
                dramp.tile([cfg.shard_g, E],
                           mybir.dt.float32 if k == cfg.nl - 1 else DT,
                           name=f"egoloc{k}")
                for k in range(cfg.nl)
            ]
            xsh = [
                dramp.tile([cfg.n_pad, E], DT, addr_space="Shared", name=f"xsh{k}")
                for k in range(cfg.nl - 1)
            ]

            xg_pools = [xgap, xgbp]
            for k in range(cfg.nl):
                table = x0 if k == 0 else xsh[k - 1]
                for t in range(cfg.tiles):
                    acc = {}
                    for m in range(2):
                        xg = xg_pools[m].tile(
                            [P, cfg.n_win, cfg.nw, E], DT,
                            name=f"xg{m}", tag=f"xg{m}")
                        idx_t = idxp.tile([P, cfg.n_win, cfg.nb8, 64],
                                          mybir.dt.int16, tag="idx", name="idx_t")
                        nc.sync.dma_start(
                            out=idx_t[:],
                            in_=gidx[m][t, :, :].rearrange(
                                "p (w b c) -> p w b c", w=cfg.n_win, b=cfg.nb8))
                        for w in range(cfg.n_win):
                            for bt in range(cfg.nb8):
                                bs = min(GB, cfg.nw - bt * GB)
                                nc.gpsimd.dma_gather(
                                    xg[:, w, bt * GB:bt * GB + bs, :],
                                    table[w * cfg.win:, :],
                                    idx_t[:, w, bt, 0:bs * 8],
                                    bs * P,
                                    bs * P,
                                    E,
                                )
                        sc_t = scalp.tile([P, S, 2], mybir.dt.float32, tag="sc", name="sc_t")
                        nc.sync.dma_start(out=sc_t[:], in_=scal[m][t, :, :, :])
                        ps = (psA if m == 0 else psB).tile(
                            [P, P], mybir.dt.float32, tag=f"ps{m}", name=f"ps{m}")
                        for w in range(cfg.n_win):
                            for jj in range(cfg.nw):
                                s = w * cfg.nw + jj
                                sv = svalp.tile([P, P], DT, tag="sv", name="sv")
                                nc.vector.tensor_scalar(
                                    out=sv[:],
                                    in0=iota_t[:],
                                    scalar1=sc_t[:, s, 0:1],
                                    scalar2=sc_t[:, s, 1:2],
                                    op0=mybir.AluOpType.is_equal,
                                    op1=mybir.AluOpType.mult,
                                )
                                nc.tensor.matmul(
                                    ps[:],
                                    lhsT=xg[:, w, jj, :],
                                    rhs=sv[:],
                                    start=(s == 0),
                                    stop=(s == S - 1),
                                )
                        a = accp.tile([P, P], mybir.dt.float32, tag=f"acc{m}",
                                      name=f"a{m}")
                        nc.scalar.copy(out=a[:], in_=ps[:])
                        acc[m] = a
                    eps = psE.tile([P, P], mybir.dt.float32, tag="eps", name="eps")
                    nc.tensor.matmul(
                        eps[:], lhsT=acc[0][:], rhs=w_t[:, k, 0, :],
                        start=True, stop=False)
                    nc.tensor.matmul(
                        eps[:], lhsT=acc[1][:], rhs=w_t[:, k, 1, :],
                        start=False, stop=True)
                    odt = mybir.dt.float32 if k == cfg.nl - 1 else DT
                    eg = egop.tile([P, P], odt, tag="eg", name="eg")
                    nc.vector.tensor_tensor(
                        out=eg[:], in0=eps[:], in1=b_t[:, k, :],
                        op=mybir.AluOpType.add)
                    nc.sync.dma_start(
                        out=ego_loc[k][t * P:(t + 1) * P, :], in_=eg[:])
                if k < cfg.nl - 1:
                    nc.gpsimd.collective_compute(
                        "AllGather",
                        mybir.AluOpType.bypass,
                        replica_groups=rg,
                        ins=[ego_loc[k][:, :]],
                        outs=[xsh[k][:, :]],
                    )

            # final batch lookup: gather rows of ego_loc[-1] then scatter to out
            fidx_t = finp.tile([P, cfg.bq // 16], mybir.dt.int16)
            nc.sync.dma_start(out=fidx_t[:], in_=fidx[:, :])
            fpos_t = finp.tile([P, cfg.bq // P], mybir.dt.int32)
            nc.sync.dma_start(out=fpos_t[:], in_=fpos[:, :])
            fg = finp.tile([P, cfg.bq // P, E], mybir.dt.float32)
            for fb in range(0, cfg.bq, GB * P):
                bs = min(GB * P, cfg.bq - fb) // P
                nc.gpsimd.dma_gather(
                    fg[:, fb // P:fb // P + bs, :],
                    ego_loc[cfg.nl - 1][:, :],
                    fidx_t[:, fb // 16:(fb + bs * P) // 16],
                    bs * P, bs * P, E)
            for j in range(cfg.bq // P):
                nc.gpsimd.indirect_dma_start(
                    out=out_d[:, :],
                    out_offset=bass.IndirectOffsetOnAxis(ap=fpos_t[:, j:j + 1], axis=0),
                    in_=fg[:, j, :],
                    in_offset=None,
                )

    nc.compile()
    return nc


# ---------------------------------------------------------------- host side


def _slot_layout(pj: np.ndarray) -> np.ndarray:
    """pj: [..., J, 128] int16 — value for gather slot (p, j) within ONE
    instruction. Returns [..., 128, J*8] image: image[..., p%16, j*8+p//16] =
    pj[..., j, p], replicated 8x over partitions."""
    J = pj.shape[-2]
    v = pj.reshape(*pj.shape[:-2], J, 8, 16)          # [..., j, a, r]
    nd = v.ndim
    img = v.transpose(*range(nd - 3), nd - 1, nd - 3, nd - 2)  # [..., r, j, a]
    img = img.reshape(*pj.shape[:-2], 16, J * 8)
    return np.tile(img, (1,) * (img.ndim - 2) + (8, 1))


def preprocess(cfg: Cfg, user_emb, item_emb, adj_val, hp_val, W, b,
               adj_row, adj_col, hp_row, hp_col, users, items):
    n_nodes = cfg.n_users + cfg.n_items
    S = cfg.slots
    npdt = cfg.npdt

    def gmap(r):
        return (r // cfg.shard) * cfg.shard_g + (r % cfg.shard)

    ego0 = np.concatenate([np.asarray(user_emb), np.asarray(item_emb)], axis=0)
    x0 = np.zeros((cfg.n_pad, E), np.float32)
    x0[gmap(np.arange(n_nodes))] = ego0
    x0 = x0.astype(npdt)

    mats = [
        (np.asarray(adj_row).astype(np.int64), np.asarray(adj_col).astype(np.int64),
         np.asarray(adj_val).astype(np.float32)),
        (np.asarray(hp_row).astype(np.int64), np.asarray(hp_col).astype(np.int64),
         np.asarray(hp_val).astype(np.float32)),
    ]

    per_core = [dict(gidx=[], scal=[]) for _ in range(cfg.n_cores)]
    cap = cfg.nw * P

    for m, (row, col, val) in enumerate(mats):
        owner = row // cfg.shard
        gcol_all = gmap(col)
        for c in range(cfg.n_cores):
            sel = owner == c
            r_loc = row[sel] - c * cfg.shard
            gcol = gcol_all[sel]
            v = val[sel]
            tile = r_loc // P
            rowloc = (r_loc % P).astype(np.float32)
            win = gcol // cfg.win
            idx16 = (gcol - win * cfg.win).astype(np.int16)

            key = (tile * cfg.n_win + win).astype(np.int64)
            order = np.argsort(key, kind="stable")
            key_s = key[order]
            cnt = np.bincount(key_s, minlength=cfg.tiles * cfg.n_win)
            if cnt.max() > cap:
                raise ValueError(f"nw too small: max count {cnt.max()} > {cap}")
            starts = np.zeros_like(cnt)
            starts[1:] = np.cumsum(cnt)[:-1]
            rank = np.arange(key_s.size) - starts[key_s]

            slot_idx = np.zeros((cfg.tiles * cfg.n_win, cap), np.int16)
            slot_rl = np.zeros((cfg.tiles * cfg.n_win, cap), np.float32)
            slot_v = np.zeros((cfg.tiles * cfg.n_win, cap), np.float32)
            slot_idx[key_s, rank] = idx16[order]
            slot_rl[key_s, rank] = rowloc[order]
            slot_v[key_s, rank] = v[order]

            # gather idx input [tiles, 128, n_win*nb8*64]
            # pad chunk count to nb8*GB per window for the slot-image build,
            # then keep only the leading 64*ceil(bs/?) columns per batch (the
            # device reads [0:bs*8] of each 64-column batch block).
            si = slot_idx.reshape(cfg.tiles, cfg.n_win, cfg.nw, P)
            padw = cfg.nb8 * GB - cfg.nw
            if padw:
                si = np.concatenate(
                    [si, np.zeros((cfg.tiles, cfg.n_win, padw, P), np.int16)],
                    axis=2)
            si = si.reshape(cfg.tiles, cfg.n_win * cfg.nb8, GB, P)
            gi = _slot_layout(si)                      # [tiles, wb, 128, 64]
            gi = gi.transpose(0, 2, 1, 3).reshape(cfg.tiles, P, cfg.n_win * cfg.nb8 * 64)
            per_core[c]["gidx"].append(np.ascontiguousarray(gi))

            rl3 = slot_rl.reshape(cfg.tiles, S, P)
            v3 = slot_v.reshape(cfg.tiles, S, P)
            sc = np.stack([rl3.transpose(0, 2, 1), v3.transpose(0, 2, 1)], axis=-1)
            per_core[c]["scal"].append(np.ascontiguousarray(sc))

    Wn = np.asarray(W).astype(np.float32)
    wt = np.stack([Wn[:, :P, :], Wn[:, P:, :]], axis=1).transpose(2, 0, 1, 3)
    wt = np.ascontiguousarray(wt)
    bn = np.asarray(b).astype(np.float32)
    bbn = np.ascontiguousarray(
        np.broadcast_to(bn[None, :, :], (P, cfg.nl, E)).astype(np.float32))
    iota = np.ascontiguousarray(
        np.broadcast_to(np.arange(P, dtype=np.float32), (P, P)))

    users = np.asarray(users).astype(np.int64)
    items = np.asarray(items).astype(np.int64)
    grow = np.concatenate([users, cfg.n_users + items])
    pos = np.arange(grow.size)
    fowner = grow // cfg.shard
    in_maps = []
    aux = dict(fowner=fowner)
    for c in range(cfg.n_cores):
        sel = fowner == c
        lrow = (grow[sel] - c * cfg.shard).astype(np.int16)
        ppos = pos[sel].astype(np.int32)
        cnt = lrow.size
        if cnt > cfg.bq:
            raise ValueError(f"bq too small: {cnt}")
        li = np.zeros(cfg.bq, np.int16)
        lp = np.full(cfg.bq, 2 * cfg.nb, np.int32) + np.arange(cfg.bq) % 8
        li[:cnt] = lrow
        lp[:cnt] = ppos
        # final gather runs in batches of GB*P idxs: build image per batch
        lib = li.reshape(-1, GB * P) if cfg.bq % (GB * P) == 0 else None
        if lib is None:
            nfull = cfg.bq // (GB * P)
            parts = [li[i * GB * P:(i + 1) * GB * P].reshape(GB, P)
                     for i in range(nfull)]
            rest = li[nfull * GB * P:]
            fimg = [_slot_layout(q) for q in parts]
            if rest.size:
                fimg.append(_slot_layout(rest.reshape(-1, P)))
            fidx = np.concatenate(fimg, axis=1)
        else:
            fidx = np.concatenate(
                [_slot_layout(q.reshape(GB, P)) for q in lib], axis=1)
        fpos = lp.reshape(cfg.bq // P, P).T.copy()
        in_maps.append(dict(
            x0=x0,
            gidx0=per_core[c]["gidx"][0], gidx1=per_core[c]["gidx"][1],
            scal0=per_core[c]["scal"][0], scal1=per_core[c]["scal"][1],
            wt=wt, bb=bbn, iota=iota,
            fidx=np.ascontiguousarray(fidx), fpos=np.ascontiguousarray(fpos),
        ))
    return in_maps, aux


def postprocess(cfg: Cfg, results, aux):
    acc = np.zeros((cfg.nout, E), np.float32)
    fowner = aux["fowner"]
    for c, r in enumerate(results):
        sel = fowner == c
        acc[:2 * cfg.nb][sel] = r["out"][:2 * cfg.nb][sel]
    return acc[:cfg.nb].copy(), acc[cfg.nb:2 * cfg.nb].copy()


_CACHE = {}


def _get_nc(cfg: Cfg):
    if cfg not in _CACHE:
        _CACHE[cfg] = build_nc(cfg)
    return _CACHE[cfg]


def make_cfg(use_bf16=True, n_users=100000, n_items=50000,
             n_cores=8, nl=3, nb=4096):
    shard = (n_users + n_items) // n_cores
    tiles = -(-shard // P)
    n_pad = tiles * P * n_cores
    n_win = -(-n_pad // WIN_MAX)
    win = -(-(-(-n_pad // n_win)) // P) * P
    return Cfg(n_users=n_users, n_items=n_items, n_cores=n_cores, shard=shard,
               tiles=tiles, nw=0, n_win=n_win, win=win, nl=nl,
               bq=0, nb=nb, use_bf16=use_bf16)


def compute_nw_bq(cfg, mats_rc, users, items):
    nw = 1
    for row, col in mats_rc:
        owner = row // cfg.shard
        gcol = (col // cfg.shard) * cfg.shard_g + (col % cfg.shard)
        win = gcol // cfg.win
        tile = (row - owner * cfg.shard) // P
        key = (owner * cfg.tiles + tile) * cfg.n_win + win
        cnt = np.bincount(key, minlength=cfg.n_cores * cfg.tiles * cfg.n_win)
        nw = max(nw, int(-(-int(cnt.max()) // P)))
    grow = np.concatenate([users, cfg.n_users + items])
    fcnt = np.bincount(grow // cfg.shard, minlength=cfg.n_cores)
    bq = int(-(-int(fcnt.max()) // P) * P)
    return nw, bq


def run(cfg, inputs, trace=False):
    nc = _get_nc(cfg)
    in_maps, aux = preprocess(cfg, **inputs)
    res = bass_utils.run_bass_kernel_spmd(
        nc, in_maps, core_ids=list(range(cfg.n_cores)), trace=trace)
    user_out, item_out = postprocess(cfg, res.results, aux)
    return (user_out, item_out), res


def kernel(user_emb, item_emb, adj_val, hp_val, W, b,
           adj_row, adj_col, hp_row, hp_col, users, items):
    use_bf16 = os.environ.get("LGCN_F32", "") != "1"
    adj_row = np.asarray(adj_row).astype(np.int64)
    adj_col = np.asarray(adj_col).astype(np.int64)
    hp_row = np.asarray(hp_row).astype(np.int64)
    hp_col = np.asarray(hp_col).astype(np.int64)
    users_n = np.asarray(users).astype(np.int64)
    items_n = np.asarray(items).astype(np.int64)

    cfg0 = make_cfg(use_bf16=use_bf16, nb=len(users_n))
    nw, bq = compute_nw_bq(cfg0, [(adj_row, adj_col), (hp_row, hp_col)],
                           users_n, items_n)
    cfg = dataclasses.replace(cfg0, nw=nw, bq=bq)

    (user_out, item_out), _ = run(cfg, dict(
        user_emb=user_emb, item_emb=item_emb, adj_val=adj_val, hp_val=hp_val,
        W=W, b=b, adj_row=adj_row, adj_col=adj_col, hp_row=hp_row,
        hp_col=hp_col, users=users_n, items=items_n))
    return user_out, item_out



# revision 6
# speedup vs baseline: 2.3380x; 2.3380x over previous
"""LGCN encoder (3-layer, dual-adjacency message passing) on 8 Trainium2 cores.

Strategy: 1D row partition of the N=150k node dimension across 8 cores.
Each SpMM is computed destination-tile by destination-tile (128 rows):
 - host pre-sorts each core's edges by (dst tile, source window), padded to a
   uniform chunk grid; chunks are 128 edges
 - device gathers source rows with dma_gather (int16 indices relative to a
   static window base; windows cover the padded node table; <=1024 idx/instr)
 - a DVE tensor_scalar builds the selector S[e, r] = (iota[r]==rowloc[e])*val[e]
 - PE accumulates X_g.T @ S into PSUM [feat, rows] over all chunks of the tile
 - the per-layer Linear(2E->E) consumes the [feat, rows] tiles directly as lhsT
Between layers an AllGather shares each core's ego shard. Final user/item
lookup is an on-device gather + indirect scatter; host merges by row ownership.
"""
import sys
import os
import dataclasses

sys.path.insert(0, "/opt/trn_rl_repo")

import numpy as np
import ml_dtypes
from dataclasses import dataclass

import concourse.bass as bass
import concourse.bacc as bacc
import concourse.mybir as mybir
from concourse.tile import TileContext
from concourse import bass_utils

P = 128
E = 128
WIN_MAX = 30720
GB = 8  # max chunks (of 128 idxs) per dma_gather instruction (1024-idx cap)


@dataclass(frozen=True)
class Cfg:
    n_users: int
    n_items: int
    n_cores: int
    shard: int          # real rows per core (n_nodes / n_cores)
    tiles: int          # tile count per core
    nw: int             # chunks (of 128 edges) per (tile, window)
    n_win: int          # source windows
    win: int            # window size in rows (<= 32767)
    nl: int             # layers
    bq: int             # final-lookup slots per core (multiple of 128)
    nb: int             # lookup batch size (users==items batch)
    use_bf16: bool

    @property
    def shard_g(self):
        return self.tiles * P

    @property
    def n_pad(self):
        return self.shard_g * self.n_cores

    @property
    def nb8(self):      # gather batches per (tile, window)
        return -(-self.nw // GB)

    @property
    def slots(self):    # chunk slots per tile per matrix
        return self.n_win * self.nw

    @property
    def dt(self):
        return mybir.dt.bfloat16 if self.use_bf16 else mybir.dt.float32

    @property
    def npdt(self):
        return ml_dtypes.bfloat16 if self.use_bf16 else np.float32

    @property
    def nout(self):
        return 2 * self.nb + 8


def build_nc(cfg: Cfg):
    DT = cfg.dt
    S = cfg.slots
    IL = cfg.n_win * cfg.nb8 * 64        # idx columns per tile

    nc = bacc.Bacc("TRN2", target_bir_lowering=False, num_swdge_queues=4)

    x0 = nc.dram_tensor("x0", [cfg.n_pad, E], DT, kind="ExternalInput")
    gidx = [
        nc.dram_tensor(f"gidx{m}", [cfg.tiles, P, IL], mybir.dt.int16,
                       kind="ExternalInput")
        for m in range(2)
    ]
    scal = [
        nc.dram_tensor(f"scal{m}", [cfg.tiles, P, S, 2], mybir.dt.float32, kind="ExternalInput")
        for m in range(2)
    ]
    wt = nc.dram_tensor("wt", [P, cfg.nl, 2, E], mybir.dt.float32, kind="ExternalInput")
    bb = nc.dram_tensor("bb", [P, cfg.nl, E], mybir.dt.float32, kind="ExternalInput")
    iota_in = nc.dram_tensor("iota", [P, P], mybir.dt.bfloat16, kind="ExternalInput")
    fidx = nc.dram_tensor("fidx", [P, cfg.bq // 16], mybir.dt.int16, kind="ExternalInput")
    fpos = nc.dram_tensor("fpos", [P, cfg.bq // P], mybir.dt.int32, kind="ExternalInput")
    out_d = nc.dram_tensor("out", [cfg.nout, E], mybir.dt.float32,
                           kind="ExternalOutput")

    rg = [list(range(cfg.n_cores))]

    with TileContext(nc) as tc:
        with (
            tc.tile_pool(name="const", bufs=1) as constp,
            tc.tile_pool(name="idxp", bufs=4) as idxp,
            tc.tile_pool(name="scalp", bufs=6) as scalp,
            tc.tile_pool(name="xga", bufs=3) as xgap,
            tc.tile_pool(name="xgb", bufs=3) as xgbp,
            tc.tile_pool(name="svalp", bufs=8) as svalp,
            tc.tile_pool(name="accp", bufs=6) as accp,
            tc.tile_pool(name="egop", bufs=4) as egop,
            tc.tile_pool(name="psA", bufs=2, space="PSUM") as psA,
            tc.tile_pool(name="psB", bufs=2, space="PSUM") as psB,
            tc.tile_pool(name="psE", bufs=2, space="PSUM") as psE,
            tc.tile_pool(name="dram", bufs=1, space="DRAM") as dramp,
            tc.tile_pool(name="fin", bufs=1) as finp,
        ):
            iota_t = constp.tile([P, P], mybir.dt.bfloat16)
            nc.sync.dma_start(out=iota_t[:], in_=iota_in[:, :])
            w_t = constp.tile([P, cfg.nl, 2, E], mybir.dt.float32)
            nc.sync.dma_start(out=w_t[:], in_=wt[:, :, :, :])
            b_t = constp.tile([P, cfg.nl, E], mybir.dt.float32)
            nc.sync.dma_start(out=b_t[:], in_=bb[:, :, :])

            ego_loc = [
                dramp.tile([cfg.shard_g, E],
                           mybir.dt.float32 if k == cfg.nl - 1 else DT,
                           name=f"egoloc{k}")
                for k in range(cfg.nl)
            ]
            xsh = [
                dramp.tile([cfg.n_pad, E], DT, addr_space="Shared", name=f"xsh{k}")
                for k in range(cfg.nl - 1)
            ]

            xg_pools = [xgap, xgbp]
            for k in range(cfg.nl):
                table = x0 if k == 0 else xsh[k - 1]
                for t in range(cfg.tiles):
                    acc = {}
                    for m in range(2):
                        xg = xg_pools[m].tile(
                            [P, cfg.n_win, cfg.nw, E], DT,
                            name=f"xg{m}", tag=f"xg{m}")
                        idx_t = idxp.tile([P, cfg.n_win, cfg.nb8, 64],
                                          mybir.dt.int16, tag="idx", name="idx_t")
                        nc.sync.dma_start(
                            out=idx_t[:],
                            in_=gidx[m][t, :, :].rearrange(
                                "p (w b c) -> p w b c", w=cfg.n_win, b=cfg.nb8))
                        for w in range(cfg.n_win):
                            for bt in range(cfg.nb8):
                                bs = min(GB, cfg.nw - bt * GB)
                                nc.gpsimd.dma_gather(
                                    xg[:, w, bt * GB:bt * GB + bs, :],
                                    table[w * cfg.win:, :],
                                    idx_t[:, w, bt, 0:bs * 8],
                                    bs * P,
                                    bs * P,
                                    E,
                                    queue_num=(t * cfg.n_win * cfg.nb8 * 2
                                               + m * cfg.n_win * cfg.nb8
                                               + w * cfg.nb8 + bt) % 4,
                                )
                        sc_t = scalp.tile([P, S, 2], mybir.dt.float32, tag="sc", name="sc_t")
                        nc.sync.dma_start(out=sc_t[:], in_=scal[m][t, :, :, :])
                        ps = (psA if m == 0 else psB).tile(
                            [P, P], mybir.dt.float32, tag=f"ps{m}", name=f"ps{m}")
                        for w in range(cfg.n_win):
                            for jj in range(cfg.nw):
                                s = w * cfg.nw + jj
                                sv = svalp.tile([P, P], DT, tag="sv", name="sv")
                                nc.vector.tensor_scalar(
                                    out=sv[:],
                                    in0=iota_t[:],
                                    scalar1=sc_t[:, s, 0:1],
                                    scalar2=sc_t[:, s, 1:2],
                                    op0=mybir.AluOpType.is_equal,
                                    op1=mybir.AluOpType.mult,
                                )
                                nc.tensor.matmul(
                                    ps[:],
                                    lhsT=xg[:, w, jj, :],
                                    rhs=sv[:],
                                    start=(s == 0),
                                    stop=(s == S - 1),
                                )
                        a = accp.tile([P, P], mybir.dt.float32, tag=f"acc{m}",
                                      name=f"a{m}")
                        nc.scalar.copy(out=a[:], in_=ps[:])
                        acc[m] = a
                    eps = psE.tile([P, P], mybir.dt.float32, tag="eps", name="eps")
                    nc.tensor.matmul(
                        eps[:], lhsT=acc[0][:], rhs=w_t[:, k, 0, :],
                        start=True, stop=False)
                    nc.tensor.matmul(
                        eps[:], lhsT=acc[1][:], rhs=w_t[:, k, 1, :],
                        start=False, stop=True)
                    odt = mybir.dt.float32 if k == cfg.nl - 1 else DT
                    eg = egop.tile([P, P], odt, tag="eg", name="eg")
                    nc.vector.tensor_tensor(
                        out=eg[:], in0=eps[:], in1=b_t[:, k, :],
                        op=mybir.AluOpType.add)
                    nc.sync.dma_start(
                        out=ego_loc[k][t * P:(t + 1) * P, :], in_=eg[:])
                if k < cfg.nl - 1:
                    nc.gpsimd.collective_compute(
                        "AllGather",
                        mybir.AluOpType.bypass,
                        replica_groups=rg,
                        ins=[ego_loc[k][:, :]],
                        outs=[xsh[k][:, :]],
                    )

            # final batch lookup: gather rows of ego_loc[-1] then scatter to out
            fidx_t = finp.tile([P, cfg.bq // 16], mybir.dt.int16)
            nc.sync.dma_start(out=fidx_t[:], in_=fidx[:, :])
            fpos_t = finp.tile([P, cfg.bq // P], mybir.dt.int32)
            nc.sync.dma_start(out=fpos_t[:], in_=fpos[:, :])
            fg = finp.tile([P, cfg.bq // P, E], mybir.dt.float32)
            for fb in range(0, cfg.bq, GB * P):
                bs = min(GB * P, cfg.bq - fb) // P
                nc.gpsimd.dma_gather(
                    fg[:, fb // P:fb // P + bs, :],
                    ego_loc[cfg.nl - 1][:, :],
                    fidx_t[:, fb // 16:(fb + bs * P) // 16],
                    bs * P, bs * P, E)
            for j in range(cfg.bq // P):
                nc.gpsimd.indirect_dma_start(
                    out=out_d[:, :],
                    out_offset=bass.IndirectOffsetOnAxis(ap=fpos_t[:, j:j + 1], axis=0),
                    in_=fg[:, j, :],
                    in_offset=None,
                )

    nc.compile()
    return nc


# ---------------------------------------------------------------- host side


def _slot_layout(pj: np.ndarray) -> np.ndarray:
    """pj: [..., J, 128] int16 — value for gather slot (p, j) within ONE
    instruction. Returns [..., 128, J*8] image: image[..., p%16, j*8+p//16] =
    pj[..., j, p], replicated 8x over partitions."""
    J = pj.shape[-2]
    v = pj.reshape(*pj.shape[:-2], J, 8, 16)          # [..., j, a, r]
    nd = v.ndim
    img = v.transpose(*range(nd - 3), nd - 1, nd - 3, nd - 2)  # [..., r, j, a]
    img = img.reshape(*pj.shape[:-2], 16, J * 8)
    return np.tile(img, (1,) * (img.ndim - 2) + (8, 1))


def preprocess(cfg: Cfg, user_emb, item_emb, adj_val, hp_val, W, b,
               adj_row, adj_col, hp_row, hp_col, users, items):
    n_nodes = cfg.n_users + cfg.n_items
    S = cfg.slots
    npdt = cfg.npdt

    def gmap(r):
        return (r // cfg.shard) * cfg.shard_g + (r % cfg.shard)

    ego0 = np.concatenate([np.asarray(user_emb), np.asarray(item_emb)], axis=0)
    x0 = np.zeros((cfg.n_pad, E), np.float32)
    x0[gmap(np.arange(n_nodes))] = ego0
    x0 = x0.astype(npdt)

    mats = [
        (np.asarray(adj_row).astype(np.int64), np.asarray(adj_col).astype(np.int64),
         np.asarray(adj_val).astype(np.float32)),
        (np.asarray(hp_row).astype(np.int64), np.asarray(hp_col).astype(np.int64),
         np.asarray(hp_val).astype(np.float32)),
    ]

    per_core = [dict(gidx=[], scal=[]) for _ in range(cfg.n_cores)]
    cap = cfg.nw * P

    for m, (row, col, val) in enumerate(mats):
        owner = row // cfg.shard
        gcol_all = gmap(col)
        for c in range(cfg.n_cores):
            sel = owner == c
            r_loc = row[sel] - c * cfg.shard
            gcol = gcol_all[sel]
            v = val[sel]
            tile = r_loc // P
            rowloc = (r_loc % P).astype(np.float32)
            win = gcol // cfg.win
            idx16 = (gcol - win * cfg.win).astype(np.int16)

            key = (tile * cfg.n_win + win).astype(np.int64)
            order = np.argsort(key, kind="stable")
            key_s = key[order]
            cnt = np.bincount(key_s, minlength=cfg.tiles * cfg.n_win)
            if cnt.max() > cap:
                raise ValueError(f"nw too small: max count {cnt.max()} > {cap}")
            starts = np.zeros_like(cnt)
            starts[1:] = np.cumsum(cnt)[:-1]
            rank = np.arange(key_s.size) - starts[key_s]

            slot_idx = np.zeros((cfg.tiles * cfg.n_win, cap), np.int16)
            slot_rl = np.zeros((cfg.tiles * cfg.n_win, cap), np.float32)
            slot_v = np.zeros((cfg.tiles * cfg.n_win, cap), np.float32)
            slot_idx[key_s, rank] = idx16[order]
            slot_rl[key_s, rank] = rowloc[order]
            slot_v[key_s, rank] = v[order]

            # gather idx input [tiles, 128, n_win*nb8*64]
            # pad chunk count to nb8*GB per window for the slot-image build,
            # then keep only the leading 64*ceil(bs/?) columns per batch (the
            # device reads [0:bs*8] of each 64-column batch block).
            si = slot_idx.reshape(cfg.tiles, cfg.n_win, cfg.nw, P)
            padw = cfg.nb8 * GB - cfg.nw
            if padw:
                si = np.concatenate(
                    [si, np.zeros((cfg.tiles, cfg.n_win, padw, P), np.int16)],
                    axis=2)
            si = si.reshape(cfg.tiles, cfg.n_win * cfg.nb8, GB, P)
            gi = _slot_layout(si)                      # [tiles, wb, 128, 64]
            gi = gi.transpose(0, 2, 1, 3).reshape(cfg.tiles, P, cfg.n_win * cfg.nb8 * 64)
            per_core[c]["gidx"].append(np.ascontiguousarray(gi))

            rl3 = slot_rl.reshape(cfg.tiles, S, P)
            v3 = slot_v.reshape(cfg.tiles, S, P)
            sc = np.stack([rl3.transpose(0, 2, 1), v3.transpose(0, 2, 1)], axis=-1)
            per_core[c]["scal"].append(np.ascontiguousarray(sc))

    Wn = np.asarray(W).astype(np.float32)
    wt = np.stack([Wn[:, :P, :], Wn[:, P:, :]], axis=1).transpose(2, 0, 1, 3)
    wt = np.ascontiguousarray(wt)
    bn = np.asarray(b).astype(np.float32)
    bbn = np.ascontiguousarray(
        np.broadcast_to(bn[None, :, :], (P, cfg.nl, E)).astype(np.float32))
    iota = np.ascontiguousarray(
        np.broadcast_to(np.arange(P, dtype=np.float32), (P, P)).astype(
            ml_dtypes.bfloat16))

    users = np.asarray(users).astype(np.int64)
    items = np.asarray(items).astype(np.int64)
    grow = np.concatenate([users, cfg.n_users + items])
    pos = np.arange(grow.size)
    fowner = grow // cfg.shard
    in_maps = []
    aux = dict(fowner=fowner)
    for c in range(cfg.n_cores):
        sel = fowner == c
        lrow = (grow[sel] - c * cfg.shard).astype(np.int16)
        ppos = pos[sel].astype(np.int32)
        cnt = lrow.size
        if cnt > cfg.bq:
            raise ValueError(f"bq too small: {cnt}")
        li = np.zeros(cfg.bq, np.int16)
        lp = np.full(cfg.bq, 2 * cfg.nb, np.int32) + np.arange(cfg.bq) % 8
        li[:cnt] = lrow
        lp[:cnt] = ppos
        # final gather runs in batches of GB*P idxs: build image per batch
        lib = li.reshape(-1, GB * P) if cfg.bq % (GB * P) == 0 else None
        if lib is None:
            nfull = cfg.bq // (GB * P)
            parts = [li[i * GB * P:(i + 1) * GB * P].reshape(GB, P)
                     for i in range(nfull)]
            rest = li[nfull * GB * P:]
            fimg = [_slot_layout(q) for q in parts]
            if rest.size:
                fimg.append(_slot_layout(rest.reshape(-1, P)))
            fidx = np.concatenate(fimg, axis=1)
        else:
            fidx = np.concatenate(
                [_slot_layout(q.reshape(GB, P)) for q in lib], axis=1)
        fpos = lp.reshape(cfg.bq // P, P).T.copy()
        in_maps.append(dict(
            x0=x0,
            gidx0=per_core[c]["gidx"][0], gidx1=per_core[c]["gidx"][1],
            scal0=per_core[c]["scal"][0], scal1=per_core[c]["scal"][1],
            wt=wt, bb=bbn, iota=iota,
            fidx=np.ascontiguousarray(fidx), fpos=np.ascontiguousarray(fpos),
        ))
    return in_maps, aux


def postprocess(cfg: Cfg, results, aux):
    acc = np.zeros((cfg.nout, E), np.float32)
    fowner = aux["fowner"]
    for c, r in enumerate(results):
        sel = fowner == c
        acc[:2 * cfg.nb][sel] = r["out"][:2 * cfg.nb][sel]
    return acc[:cfg.nb].copy(), acc[cfg.nb:2 * cfg.nb].copy()


_CACHE = {}


def _get_nc(cfg: Cfg):
    if cfg not in _CACHE:
        _CACHE[cfg] = build_nc(cfg)
    return _CACHE[cfg]


def make_cfg(use_bf16=True, n_users=100000, n_items=50000,
             n_cores=8, nl=3, nb=4096):
    shard = (n_users + n_items) // n_cores
    tiles = -(-shard // P)
    n_pad = tiles * P * n_cores
    n_win = -(-n_pad // WIN_MAX)
    win = -(-(-(-n_pad // n_win)) // P) * P
    return Cfg(n_users=n_users, n_items=n_items, n_cores=n_cores, shard=shard,
               tiles=tiles, nw=0, n_win=n_win, win=win, nl=nl,
               bq=0, nb=nb, use_bf16=use_bf16)


def compute_nw_bq(cfg, mats_rc, users, items):
    nw = 1
    for row, col in mats_rc:
        owner = row // cfg.shard
        gcol = (col // cfg.shard) * cfg.shard_g + (col % cfg.shard)
        win = gcol // cfg.win
        tile = (row - owner * cfg.shard) // P
        key = (owner * cfg.tiles + tile) * cfg.n_win + win
        cnt = np.bincount(key, minlength=cfg.n_cores * cfg.tiles * cfg.n_win)
        nw = max(nw, int(-(-int(cnt.max()) // P)))
    grow = np.concatenate([users, cfg.n_users + items])
    fcnt = np.bincount(grow // cfg.shard, minlength=cfg.n_cores)
    bq = int(-(-int(fcnt.max()) // P) * P)
    return nw, bq


def run(cfg, inputs, trace=False):
    nc = _get_nc(cfg)
    in_maps, aux = preprocess(cfg, **inputs)
    res = bass_utils.run_bass_kernel_spmd(
        nc, in_maps, core_ids=list(range(cfg.n_cores)), trace=trace)
    user_out, item_out = postprocess(cfg, res.results, aux)
    return (user_out, item_out), res


def kernel(user_emb, item_emb, adj_val, hp_val, W, b,
           adj_row, adj_col, hp_row, hp_col, users, items):
    use_bf16 = os.environ.get("LGCN_F32", "") != "1"
    adj_row = np.asarray(adj_row).astype(np.int64)
    adj_col = np.asarray(adj_col).astype(np.int64)
    hp_row = np.asarray(hp_row).astype(np.int64)
    hp_col = np.asarray(hp_col).astype(np.int64)
    users_n = np.asarray(users).astype(np.int64)
    items_n = np.asarray(items).astype(np.int64)

    cfg0 = make_cfg(use_bf16=use_bf16, nb=len(users_n))
    nw, bq = compute_nw_bq(cfg0, [(adj_row, adj_col), (hp_row, hp_col)],
                           users_n, items_n)
    cfg = dataclasses.replace(cfg0, nw=nw, bq=bq)

    (user_out, item_out), _ = run(cfg, dict(
        user_emb=user_emb, item_emb=item_emb, adj_val=adj_val, hp_val=hp_val,
        W=W, b=b, adj_row=adj_row, adj_col=adj_col, hp_row=hp_row,
        hp_col=hp_col, users=users_n, items=items_n))
    return user_out, item_out



# revision 9
# speedup vs baseline: 2.7790x; 1.1886x over previous
"""LGCN encoder (3-layer, dual-adjacency message passing) on 8 Trainium2 cores.

Strategy: 1D row partition of the N=150k node dimension across 8 cores.
Each SpMM is computed destination-tile by destination-tile (128 rows):
 - host pre-sorts each core's edges by (dst tile, source window) with EXACT
   per-(tile,window) chunk counts (max across cores after per-core tile-rank
   matching), chunks are 128 edges
 - device gathers source rows with dma_gather (int16 indices relative to a
   static window base; 4 SWDGE queues round-robin; <=1024 idx/instr)
 - a DVE tensor_scalar builds the selector S[e, r] = (iota[r]==rowloc[e])*val[e]
 - PE accumulates X_g.T @ S into PSUM [feat, rows] over all chunks of the tile
 - the per-layer Linear(2E->E) consumes the [feat, rows] tiles directly as lhsT
Between layers an AllGather shares each core's ego shard. Final user/item
lookup is an on-device gather + indirect scatter; host merges by row ownership.
"""
import sys
import os
import dataclasses

sys.path.insert(0, "/opt/trn_rl_repo")

import numpy as np
import ml_dtypes
from dataclasses import dataclass

import concourse.bass as bass
import concourse.bacc as bacc
import concourse.mybir as mybir
from concourse.tile import TileContext
from concourse import bass_utils

P = 128
E = 128
WIN_MAX = 30720
GB = 8  # max chunks (of 128 idxs) per dma_gather instruction (1024-idx cap)


@dataclass(frozen=True)
class Cfg:
    n_users: int
    n_items: int
    n_cores: int
    shard: int          # real rows per core (n_nodes / n_cores)
    tiles: int          # tile count per core
    n_win: int          # source windows
    win: int            # window size in rows (<= 32767)
    nl: int             # layers
    nb: int             # lookup batch size (users==items batch)
    use_bf16: bool

    @property
    def shard_g(self):
        return self.tiles * P

    @property
    def n_pad(self):
        return self.shard_g * self.n_cores

    @property
    def dt(self):
        return mybir.dt.bfloat16 if self.use_bf16 else mybir.dt.float32

    @property
    def npdt(self):
        return ml_dtypes.bfloat16 if self.use_bf16 else np.float32

    @property
    def nout(self):
        return 2 * self.nb + 8


def build_nc(cfg: Cfg, layout):
    DT = cfg.dt
    K = layout["K"]            # [2][tiles][n_win] chunk counts
    ST = layout["ST"]          # [2] total slots per matrix
    maxS = layout["maxS"]      # [2] max slots per tile
    bq = layout["bq"]

    nc = bacc.Bacc("TRN2", target_bir_lowering=False, num_swdge_queues=4)

    x0 = nc.dram_tensor("x0", [cfg.n_pad, E], DT, kind="ExternalInput")
    gidx = [
        nc.dram_tensor(f"gidx{m}", [P, ST[m] * 8], mybir.dt.int16,
                       kind="ExternalInput")
        for m in range(2)
    ]
    scal = [
        nc.dram_tensor(f"scal{m}", [P, ST[m], 2], mybir.dt.float32,
                       kind="ExternalInput")
        for m in range(2)
    ]
    wt = nc.dram_tensor("wt", [P, cfg.nl, 2, E], mybir.dt.float32, kind="ExternalInput")
    bb = nc.dram_tensor("bb", [P, cfg.nl, E], mybir.dt.float32, kind="ExternalInput")
    iota_in = nc.dram_tensor("iota", [P, P], mybir.dt.bfloat16, kind="ExternalInput")
    fidx = nc.dram_tensor("fidx", [P, bq // 16], mybir.dt.int16, kind="ExternalInput")
    fpos = nc.dram_tensor("fpos", [P, bq // P], mybir.dt.int32, kind="ExternalInput")
    out_d = nc.dram_tensor("out", [cfg.nout, E], mybir.dt.float32,
                           kind="ExternalOutput")

    rg = [list(range(cfg.n_cores))]
    qrr = [0]

    with TileContext(nc) as tc:
        with (
            tc.tile_pool(name="const", bufs=1) as constp,
            tc.tile_pool(name="idxp", bufs=4) as idxp,
            tc.tile_pool(name="scalp", bufs=6) as scalp,
            tc.tile_pool(name="xga", bufs=3) as xgap,
            tc.tile_pool(name="xgb", bufs=3) as xgbp,
            tc.tile_pool(name="svalp", bufs=8) as svalp,
            tc.tile_pool(name="accp", bufs=6) as accp,
            tc.tile_pool(name="egop", bufs=4) as egop,
            tc.tile_pool(name="psA", bufs=2, space="PSUM") as psA,
            tc.tile_pool(name="psB", bufs=2, space="PSUM") as psB,
            tc.tile_pool(name="psE", bufs=2, space="PSUM") as psE,
            tc.tile_pool(name="dram", bufs=1, space="DRAM") as dramp,
            tc.tile_pool(name="fin", bufs=1) as finp,
        ):
            iota_t = constp.tile([P, P], mybir.dt.bfloat16)
            nc.sync.dma_start(out=iota_t[:], in_=iota_in[:, :])
            w_t = constp.tile([P, cfg.nl, 2, E], mybir.dt.float32)
            nc.sync.dma_start(out=w_t[:], in_=wt[:, :, :, :])
            b_t = constp.tile([P, cfg.nl, E], mybir.dt.float32)
            nc.sync.dma_start(out=b_t[:], in_=bb[:, :, :])

            ego_loc = [
                dramp.tile([cfg.shard_g, E],
                           mybir.dt.float32 if k == cfg.nl - 1 else DT,
                           name=f"egoloc{k}")
                for k in range(cfg.nl)
            ]
            xsh = [
                dramp.tile([cfg.n_pad, E], DT, addr_space="Shared", name=f"xsh{k}")
                for k in range(cfg.nl - 1)
            ]

            xg_pools = [xgap, xgbp]
            for k in range(cfg.nl):
                table = x0 if k == 0 else xsh[k - 1]
                ioff = [0, 0]
                soff = [0, 0]
                for t in range(cfg.tiles):
                    acc = {}
                    for m in range(2):
                        Sj = sum(K[m][t])
                        ILj = Sj * 8
                        xg = xg_pools[m].tile(
                            [P, maxS[m], E], DT,
                            name=f"xg{m}", tag=f"xg{m}")
                        idx_t = idxp.tile([P, maxS[m] * 8],
                                          mybir.dt.int16, tag="idx", name="idx_t")
                        nc.sync.dma_start(
                            out=idx_t[:, 0:ILj],
                            in_=gidx[m][:, ioff[m]:ioff[m] + ILj])
                        col = 0
                        ic = 0
                        for w in range(cfg.n_win):
                            Kw = K[m][t][w]
                            for bt in range(0, Kw, GB):
                                bs = min(GB, Kw - bt)
                                nc.gpsimd.dma_gather(
                                    xg[:, col + bt:col + bt + bs, :],
                                    table[w * cfg.win:, :],
                                    idx_t[:, ic:ic + bs * 8],
                                    bs * P,
                                    bs * P,
                                    E,
                                    queue_num=qrr[0] % 4,
                                )
                                qrr[0] += 1
                                ic += bs * 8
                            col += Kw
                        sc_t = scalp.tile([P, maxS[m], 2], mybir.dt.float32,
                                          tag="sc", name="sc_t")
                        nc.sync.dma_start(
                            out=sc_t[:, 0:Sj, :],
                            in_=scal[m][:, soff[m]:soff[m] + Sj, :])
                        ioff[m] += ILj
                        soff[m] += Sj
                        ps = (psA if m == 0 else psB).tile(
                            [P, P], mybir.dt.float32, tag=f"ps{m}", name=f"ps{m}")
                        for s in range(Sj):
                            sv = svalp.tile([P, P], DT, tag="sv", name="sv")
                            nc.vector.tensor_scalar(
                                out=sv[:],
                                in0=iota_t[:],
                                scalar1=sc_t[:, s, 0:1],
                                scalar2=sc_t[:, s, 1:2],
                                op0=mybir.AluOpType.is_equal,
                                op1=mybir.AluOpType.mult,
                            )
                            nc.tensor.matmul(
                                ps[:],
                                lhsT=xg[:, s, :],
                                rhs=sv[:],
                                start=(s == 0),
                                stop=(s == Sj - 1),
                            )
                        a = accp.tile([P, P], mybir.dt.float32, tag=f"acc{m}",
                                      name=f"a{m}")
                        nc.scalar.copy(out=a[:], in_=ps[:])
                        acc[m] = a
                    eps = psE.tile([P, P], mybir.dt.float32, tag="eps", name="eps")
                    nc.tensor.matmul(
                        eps[:], lhsT=acc[0][:], rhs=w_t[:, k, 0, :],
                        start=True, stop=False)
                    nc.tensor.matmul(
                        eps[:], lhsT=acc[1][:], rhs=w_t[:, k, 1, :],
                        start=False, stop=True)
                    odt = mybir.dt.float32 if k == cfg.nl - 1 else DT
                    eg = egop.tile([P, P], odt, tag="eg", name="eg")
                    nc.vector.tensor_tensor(
                        out=eg[:], in0=eps[:], in1=b_t[:, k, :],
                        op=mybir.AluOpType.add)
                    nc.sync.dma_start(
                        out=ego_loc[k][t * P:(t + 1) * P, :], in_=eg[:])
                if k < cfg.nl - 1:
                    nc.gpsimd.collective_compute(
                        "AllGather",
                        mybir.AluOpType.bypass,
                        replica_groups=rg,
                        ins=[ego_loc[k][:, :]],
                        outs=[xsh[k][:, :]],
                    )

            # final batch lookup: gather rows of ego_loc[-1] then scatter to out
            fidx_t = finp.tile([P, bq // 16], mybir.dt.int16)
            nc.sync.dma_start(out=fidx_t[:], in_=fidx[:, :])
            fpos_t = finp.tile([P, bq // P], mybir.dt.int32)
            nc.sync.dma_start(out=fpos_t[:], in_=fpos[:, :])
            fg = finp.tile([P, bq // P, E], mybir.dt.float32)
            for fb in range(0, bq, GB * P):
                bs = min(GB * P, bq - fb) // P
                nc.gpsimd.dma_gather(
                    fg[:, fb // P:fb // P + bs, :],
                    ego_loc[cfg.nl - 1][:, :],
                    fidx_t[:, fb // 16:(fb + bs * P) // 16],
                    bs * P, bs * P, E)
            for j in range(bq // P):
                nc.gpsimd.indirect_dma_start(
                    out=out_d[:, :],
                    out_offset=bass.IndirectOffsetOnAxis(ap=fpos_t[:, j:j + 1], axis=0),
                    in_=fg[:, j, :],
                    in_offset=None,
                )

    nc.compile()
    return nc


# ---------------------------------------------------------------- host side


def _slot_layout(pj: np.ndarray) -> np.ndarray:
    """pj: [..., J, 128] int16 — value for gather slot (p, j) within ONE
    instruction. Returns [..., 128, J*8] image: image[..., p%16, j*8+p//16] =
    pj[..., j, p], replicated 8x over partitions."""
    J = pj.shape[-2]
    v = pj.reshape(*pj.shape[:-2], J, 8, 16)          # [..., j, a, r]
    nd = v.ndim
    img = v.transpose(*range(nd - 3), nd - 1, nd - 3, nd - 2)  # [..., r, j, a]
    img = img.reshape(*pj.shape[:-2], 16, J * 8)
    return np.tile(img, (1,) * (img.ndim - 2) + (8, 1))


def compute_layout(cfg: Cfg, mats_rc, users, items):
    """Exact per-(tile,window) chunk counts, tile ranks matched across cores.

    Returns (layout, perms) where layout is hashable-ish dict for build_nc and
    perms[c] is the tile->slot permutation (slot j processes tile perm[c][j])."""
    nt, nw_, nco = cfg.tiles, cfg.n_win, cfg.n_cores
    # pass A: window-independent per-(core,tile) edge totals -> tile ranks
    total = np.zeros((nco, nt), np.int64)
    for m, (row, col) in enumerate(mats_rc):
        owner = row // cfg.shard
        tile = (row - owner * cfg.shard) // P
        total += np.bincount(owner * nt + tile,
                             minlength=nco * nt).reshape(nco, nt)
    perms = np.argsort(-total, axis=1, kind="stable")      # [nco, nt]
    invperm = np.argsort(perms, axis=1)                    # [nco, nt]
    # pass B: per-(slot,window) counts using the SAME permuted column map
    # that preprocess uses (windows depend on the permutation)
    cnt = np.zeros((2, nco, nt, nw_), np.int64)
    for m, (row, col) in enumerate(mats_rc):
        owner = row // cfg.shard
        loc = row - owner * cfg.shard
        j = invperm[owner, loc // P]
        cown = col // cfg.shard
        cloc = col - cown * cfg.shard
        gcol = cown * cfg.shard_g + invperm[cown, cloc // P] * P + cloc % P
        win = gcol // cfg.win
        key = ((owner * nt + j) * nw_ + win).astype(np.int64)
        cnt[m] = np.bincount(key, minlength=nco * nt * nw_).reshape(nco, nt, nw_)
    chunks = -(-cnt // P)                       # [2, nco, nt, nw]
    K = np.maximum(chunks.max(axis=1), 1)       # [2, nt, nw]
    ST = K.sum(axis=(1, 2))                     # [2]
    maxS = K.sum(axis=2).max(axis=1)            # [2]

    grow = np.concatenate([users, cfg.n_users + items])
    fcnt = np.bincount(grow // cfg.shard, minlength=nco)
    bq = int(-(-int(fcnt.max()) // P) * P)
    layout = dict(
        K=tuple(tuple(tuple(int(x) for x in r) for r in Km) for Km in K),
        ST=tuple(int(x) for x in ST),
        maxS=tuple(int(x) for x in maxS),
        bq=bq,
    )
    return layout, perms, invperm


def preprocess(cfg: Cfg, layout, perms, invperm, user_emb, item_emb, adj_val,
               hp_val, W, b, adj_row, adj_col, hp_row, hp_col, users, items):
    n_nodes = cfg.n_users + cfg.n_items
    npdt = cfg.npdt
    nt, nw_, nco = cfg.tiles, cfg.n_win, cfg.n_cores
    K = np.array(layout["K"], np.int64)         # [2, nt, nw]
    ST = layout["ST"]
    bq = layout["bq"]

    def gmap(r):
        c = r // cfg.shard
        loc = r - c * cfg.shard
        t = loc // P
        return c * cfg.shard_g + invperm[c, t] * P + loc % P

    ego0 = np.concatenate([np.asarray(user_emb), np.asarray(item_emb)], axis=0)
    x0 = np.zeros((cfg.n_pad, E), np.float32)
    x0[gmap(np.arange(n_nodes))] = ego0
    x0 = x0.astype(npdt)

    mats = [
        (np.asarray(adj_row).astype(np.int64), np.asarray(adj_col).astype(np.int64),
         np.asarray(adj_val).astype(np.float32)),
        (np.asarray(hp_row).astype(np.int64), np.asarray(hp_col).astype(np.int64),
         np.asarray(hp_val).astype(np.float32)),
    ]

    # global slot offsets per (m, j, w): slots laid out j-major, then window
    slotoff = np.zeros((2, nt, nw_), np.int64)
    for m in range(2):
        flat = K[m].reshape(-1)
        off = np.zeros_like(flat)
        off[1:] = np.cumsum(flat)[:-1]
        slotoff[m] = off.reshape(nt, nw_)

    per_core = [dict(gidx=[], scal=[]) for _ in range(nco)]

    for m, (row, col, val) in enumerate(mats):
        owner = row // cfg.shard
        gcol_all = gmap(col)
        for c in range(nco):
            sel = owner == c
            r_loc = row[sel] - c * cfg.shard
            gcol = gcol_all[sel]
            v = val[sel]
            tile = r_loc // P
            j = invperm[c][tile]                # tile slot rank
            rowloc = (r_loc % P).astype(np.float32)
            win = gcol // cfg.win
            idx16 = (gcol - win * cfg.win).astype(np.int16)

            key = (j * nw_ + win).astype(np.int64)
            order = np.argsort(key, kind="stable")
            key_s = key[order]
            cntk = np.bincount(key_s, minlength=nt * nw_)
            starts = np.zeros_like(cntk)
            starts[1:] = np.cumsum(cntk)[:-1]
            rank = np.arange(key_s.size) - starts[key_s]
            # global slot position = slotoff[m].flat[key]*128 + rank
            pos = slotoff[m].reshape(-1)[key_s] * P + rank

            S_m = ST[m]
            slot_idx = np.zeros(S_m * P, np.int16)
            slot_rl = np.zeros(S_m * P, np.float32)
            slot_v = np.zeros(S_m * P, np.float32)
            slot_idx[pos] = idx16[order]
            slot_rl[pos] = rowloc[order]
            slot_v[pos] = v[order]
            slot_idx = slot_idx.reshape(S_m, P)

            # gather idx image: per (j, w) group of K chunks, batches of GB
            imgs = []
            for jj in range(nt):
                for w in range(nw_):
                    Kw = int(K[m, jj, w])
                    base = int(slotoff[m, jj, w])
                    for bt in range(0, Kw, GB):
                        bs = min(GB, Kw - bt)
                        blk = slot_idx[base + bt:base + bt + bs]   # [bs, P]
                        imgs.append(_slot_layout(blk))
            gi = np.concatenate(imgs, axis=1)          # [P, ST*8]
            assert gi.shape == (P, S_m * 8), gi.shape
            per_core[c]["gidx"].append(np.ascontiguousarray(gi))

            rl3 = slot_rl.reshape(S_m, P).T            # [P, S]
            v3 = slot_v.reshape(S_m, P).T
            sc = np.stack([rl3, v3], axis=-1).astype(np.float32)
            per_core[c]["scal"].append(np.ascontiguousarray(sc))

    Wn = np.asarray(W).astype(np.float32)
    wt = np.stack([Wn[:, :P, :], Wn[:, P:, :]], axis=1).transpose(2, 0, 1, 3)
    wt = np.ascontiguousarray(wt)
    bn = np.asarray(b).astype(np.float32)
    bbn = np.ascontiguousarray(
        np.broadcast_to(bn[None, :, :], (P, cfg.nl, E)).astype(np.float32))
    iota = np.ascontiguousarray(
        np.broadcast_to(np.arange(P, dtype=np.float32), (P, P)).astype(
            ml_dtypes.bfloat16))

    users = np.asarray(users).astype(np.int64)
    items = np.asarray(items).astype(np.int64)
    grow = np.concatenate([users, cfg.n_users + items])
    pos = np.arange(grow.size)
    fowner = grow // cfg.shard
    in_maps = []
    aux = dict(fowner=fowner)
    for c in range(nco):
        sel = fowner == c
        gl = grow[sel] - c * cfg.shard
        tl = gl // P
        lrow = (invperm[c][tl] * P + gl % P).astype(np.int16)
        ppos = pos[sel].astype(np.int32)
        cnt_ = lrow.size
        if cnt_ > bq:
            raise ValueError(f"bq too small: {cnt_}")
        li = np.zeros(bq, np.int16)
        lp = np.full(bq, 2 * cfg.nb, np.int32) + np.arange(bq) % 8
        li[:cnt_] = lrow
        lp[:cnt_] = ppos
        nfull = bq // (GB * P)
        parts = [li[i * GB * P:(i + 1) * GB * P].reshape(GB, P)
                 for i in range(nfull)]
        rest = li[nfull * GB * P:]
        fimg = [_slot_layout(q) for q in parts]
        if rest.size:
            fimg.append(_slot_layout(rest.reshape(-1, P)))
        fidx = np.concatenate(fimg, axis=1)
        fpos = lp.reshape(bq // P, P).T.copy()
        in_maps.append(dict(
            x0=x0,
            gidx0=per_core[c]["gidx"][0], gidx1=per_core[c]["gidx"][1],
            scal0=per_core[c]["scal"][0], scal1=per_core[c]["scal"][1],
            wt=wt, bb=bbn, iota=iota,
            fidx=np.ascontiguousarray(fidx), fpos=np.ascontiguousarray(fpos),
        ))
    return in_maps, aux


def postprocess(cfg: Cfg, results, aux):
    acc = np.zeros((cfg.nout, E), np.float32)
    fowner = aux["fowner"]
    for c, r in enumerate(results):
        sel = fowner == c
        acc[:2 * cfg.nb][sel] = r["out"][:2 * cfg.nb][sel]
    return acc[:cfg.nb].copy(), acc[cfg.nb:2 * cfg.nb].copy()


_CACHE = {}


def _get_nc(cfg: Cfg, layout):
    key = (cfg, layout["K"], layout["bq"])
    if key not in _CACHE:
        _CACHE[key] = build_nc(cfg, layout)
    return _CACHE[key]


def make_cfg(use_bf16=True, n_users=100000, n_items=50000,
             n_cores=8, nl=3, nb=4096):
    shard = (n_users + n_items) // n_cores
    tiles = -(-shard // P)
    n_pad = tiles * P * n_cores
    n_win = -(-n_pad // WIN_MAX)
    win = -(-(-(-n_pad // n_win)) // P) * P
    return Cfg(n_users=n_users, n_items=n_items, n_cores=n_cores, shard=shard,
               tiles=tiles, n_win=n_win, win=win, nl=nl, nb=nb,
               use_bf16=use_bf16)


def run(cfg, inputs, trace=False):
    mats_rc = [
        (np.asarray(inputs["adj_row"]).astype(np.int64),
         np.asarray(inputs["adj_col"]).astype(np.int64)),
        (np.asarray(inputs["hp_row"]).astype(np.int64),
         np.asarray(inputs["hp_col"]).astype(np.int64)),
    ]
    users_n = np.asarray(inputs["users"]).astype(np.int64)
    items_n = np.asarray(inputs["items"]).astype(np.int64)
    layout, perms, invperm = compute_layout(cfg, mats_rc, users_n, items_n)
    nc = _get_nc(cfg, layout)
    in_maps, aux = preprocess(cfg, layout, perms, invperm, **inputs)
    res = bass_utils.run_bass_kernel_spmd(
        nc, in_maps, core_ids=list(range(cfg.n_cores)), trace=trace)
    user_out, item_out = postprocess(cfg, res.results, aux)
    return (user_out, item_out), res


def kernel(user_emb, item_emb, adj_val, hp_val, W, b,
           adj_row, adj_col, hp_row, hp_col, users, items):
    use_bf16 = os.environ.get("LGCN_F32", "") != "1"
    inputs = dict(
        user_emb=user_emb, item_emb=item_emb, adj_val=adj_val, hp_val=hp_val,
        W=W, b=b,
        adj_row=np.asarray(adj_row).astype(np.int64),
        adj_col=np.asarray(adj_col).astype(np.int64),
        hp_row=np.asarray(hp_row).astype(np.int64),
        hp_col=np.asarray(hp_col).astype(np.int64),
        users=np.asarray(users).astype(np.int64),
        items=np.asarray(items).astype(np.int64))
    cfg = make_cfg(use_bf16=use_bf16, nb=len(np.asarray(users)))
    (user_out, item_out), _ = run(cfg, inputs)
    return user_out, item_out
